# revision 7
# baseline (speedup 1.0000x reference)
"""Trainium2 Bass kernel for nn_Cal_adj_matrix (pyramid-pool adjacency).

Computes, per sample b:
    feature = x[b].reshape(C, M)                  # M = H*W = 9216
    pool    = pyramid_pool(x[b])                  # (C, 50), pools of size 1,2,3,6
    sim     = relu(feature^T @ pool / (B*C*H*W))  # (M, 50)
    total   = sim.sum(-1)                         # (M,)
    adj     = sim / (total^2 + 1e-6)              # (M, 50)

Sharding: data-parallel over batch; 32 samples -> 4 per core x 8 cores.

Perf structure (memory-bound, ~360 GB/s/core DMA is the roofline):
  - input is read once as fp32 (37.7 MB/core), output written as fp16
    (3.7 MB/core) and upcast on host -> DMA floor ~115 us/core/pass.
  - fp32->bf16 cast on ACT; pyramid stage-1 (sum of 16 w-neighbours) as a
    bf16 pairwise-add tree on DVE (2x perf mode) instead of a 1x reduce.
  - matmuls accumulate both 128-channel halves into PSUM banks with a
    single start=True per bank (byte-granular lazy zeroing), so the ch0
    half's matmuls dispatch while the ch1 half is still streaming in.
  - m-index mapping m = p*72 + j keeps each output DMA contiguous.
"""

import numpy as np

import concourse.bass as bass
import concourse.bacc as bacc
import concourse.mybir as mybir
import concourse.tile as tile
from concourse.bass_utils import run_bass_kernel_spmd

# Problem shape (hardcoded; kernel.py must be self-contained).
B, C, H, W = 32, 256, 96, 96
M = H * W            # 9216
N = 50               # 1 + 4 + 9 + 36 pyramid tokens
NCORES = 8
BS = B // NCORES     # 4 samples per core
DIV = float(B * C * H * W)  # reference's global divisor

FP32 = mybir.dt.float32
BF16 = mybir.dt.bfloat16
FP16 = mybir.dt.float16

# m-index mapping: m = p*72 + j  (p = SBUF/PSUM partition, j = matmul index).
# This makes each sample's output a fully-contiguous DMA per partition line.
JN = M // 128        # 72 matmul column-groups per sample

BANK_J = 9           # matmul groups per PSUM bank (9*50=450 <= 512)
NBANK = JN // BANK_J  # 8 bank groups per sample


def build_nc(reps=1, xq_bufs=6, feat_bufs=3, outb_bufs=2, nq=4,
             out_dtype=FP16, stage1="tree", chsplit=True, scale_group=2,
             out_ring="gpsimd", out_group=2, relu_eng="split"):
    QH = H // nq         # h-rows per input chunk
    QM = QH * W          # elements per chunk
    nc = bacc.Bacc(
        "TRN2",
        target_bir_lowering=False,
        debug=False,
        enable_asserts=True,
        num_devices=NCORES,
    )
    x = nc.dram_tensor("x", [BS, C, H, W], FP32, kind="ExternalInput").ap()
    out = nc.dram_tensor("out", [BS, M, N], out_dtype, kind="ExternalOutput").ap()

    # scale factors folded into the pool values: 1/(bin_elems * DIV)
    k1 = 1.0 / (9216.0 * DIV)
    k2 = 1.0 / (2304.0 * DIV)
    k3 = 1.0 / (1024.0 * DIV)
    k6 = 1.0 / (256.0 * DIV)

    out_eng = {"scalar": "scalar", "gpsimd": "gpsimd", "vector": "vector",
               "sync": "sync"}[out_ring]

    with tile.TileContext(nc) as tc:
        with (
            tc.tile_pool(name="xq", bufs=xq_bufs) as xq_pool,
            tc.tile_pool(name="featbf", bufs=feat_bufs) as feat_pool,
            tc.tile_pool(name="tree", bufs=4) as tree_pool,
            tc.tile_pool(name="r1", bufs=4) as r1_pool,
            tc.tile_pool(name="pools", bufs=8) as small_pool,
            tc.tile_pool(name="poolbf", bufs=4) as poolbf_pool,
            tc.tile_pool(name="outb", bufs=outb_bufs) as outb_pool,
            tc.tile_pool(name="outf", bufs=4) as outf_pool,
            tc.tile_pool(name="stats", bufs=4) as stats_pool,
            tc.tile_pool(name="psum", bufs=8, space="PSUM") as psum_pool,
        ):
            for rep in range(reps):
                for s in range(BS):
                    last = s == BS - 1
                    featbf = []
                    poolbf = []
                    for ch in range(2):
                        c0 = ch * 128
                        fb = feat_pool.tile([128, M], BF16, tag="featbf")
                        r1 = r1_pool.tile([128, 576], FP32, tag="r1")
                        for q in range(nq):
                            h0 = q * QH
                            t32 = xq_pool.tile([128, QM], FP32, tag="xq")
                            src = x[s, c0:c0 + 128, h0:h0 + QH, :]
                            nc.sync.dma_start(
                                out=t32[:], in_=src.rearrange("c h w -> c (h w)"))
                            # fp32 -> bf16 cast on ScalarE
                            fbs = fb[:, h0 * W:(h0 + QH) * W]
                            nc.scalar.copy(fbs, t32[:])
                            r1s = r1[:, h0 * 6:(h0 + QH) * 6]
                            # stage-1 pool: sum 16 contiguous w-elements.
                            # "tree": bf16 pairwise adds on DVE (2x mode);
                            # for the tail-critical final chunk, reduce from
                            # the fp32 tile instead so it runs concurrently
                            # with the cast.
                            tail_chunk = (last and ch == 1 and q == nq - 1
                                          and rep == reps - 1)
                            if stage1 == "tree" and not tail_chunk:
                                g = QH * 6
                                a = fbs.rearrange("p (g k) -> p g k", k=16)
                                t1 = tree_pool.tile([128, g * 8], BF16, tag="t1")
                                v1 = t1[:, :].rearrange("p (g k) -> p g k", k=8)
                                nc.vector.tensor_add(v1, a[:, :, 0:8], a[:, :, 8:16])
                                t2 = tree_pool.tile([128, g * 4], BF16, tag="t2")
                                v2 = t2[:, :].rearrange("p (g k) -> p g k", k=4)
                                nc.vector.tensor_add(v2, v1[:, :, 0:4], v1[:, :, 4:8])
                                t3 = tree_pool.tile([128, g * 2], BF16, tag="t3")
                                v3 = t3[:, :].rearrange("p (g k) -> p g k", k=2)
                                nc.vector.tensor_add(v3, v2[:, :, 0:2], v2[:, :, 2:4])
                                nc.vector.tensor_add(r1s, v3[:, :, 0], v3[:, :, 1])
                            else:
                                nc.vector.reduce_sum(
                                    r1s,
                                    t32[:, :].rearrange("p (g k) -> p g k", k=16),
                                    axis=mybir.AxisListType.X,
                                )
                        # stage-2: A[hb,wb] = 16x16 block sums.  r1 free idx =
                        # h*6+wb, h = hb*16+hh  ->  idx = hb*96 + hh*6 + wb
                        A = small_pool.tile([128, 36], FP32, tag="A")
                        nc.vector.reduce_sum(
                            A[:, :],
                            r1[:, :576].rearrange(
                                "p (hb hh wb) -> p hb wb hh", hb=6, hh=16, wb=6),
                            axis=mybir.AxisListType.X,
                        )
                        # s=3 pools: 2x2 groups of A blocks
                        Bt = small_pool.tile([128, 18], FP32, tag="B")
                        a2 = A[:, :36].rearrange(
                            "p (hb wp t) -> p t hb wp", hb=6, wp=3, t=2)
                        nc.vector.tensor_add(Bt[:, :], a2[:, 0, :], a2[:, 1, :])
                        s3raw = small_pool.tile([128, 9], FP32, tag="s3")
                        b2 = Bt[:, :18].rearrange(
                            "p (hp t wp) -> p t hp wp", hp=3, t=2, wp=3)
                        nc.vector.tensor_add(s3raw[:, :], b2[:, 0, :], b2[:, 1, :])
                        # s=2 pools: 3x3 groups of A blocks
                        Ct = small_pool.tile([128, 12], FP32, tag="C")
                        nc.vector.reduce_sum(
                            Ct[:, :],
                            A[:, :36].rearrange(
                                "p (hb wq wt) -> p (hb wq) wt", hb=6, wq=2, wt=3),
                            axis=mybir.AxisListType.X,
                        )
                        s2raw = small_pool.tile([128, 4], FP32, tag="s2")
                        nc.vector.reduce_sum(
                            s2raw[:, :],
                            Ct[:, :12].rearrange(
                                "p (hq ht wq) -> p hq wq ht", hq=2, ht=3, wq=2),
                            axis=mybir.AxisListType.X,
                        )
                        # s=1 pool
                        s1raw = small_pool.tile([128, 1], FP32, tag="s1")
                        nc.vector.reduce_sum(
                            s1raw[:, :], A[:, :36], axis=mybir.AxisListType.X)

                        pb = poolbf_pool.tile([128, N], BF16, tag="poolbf")
                        nc.vector.tensor_scalar_mul(pb[:, 0:1], s1raw[:, :], k1)
                        nc.vector.tensor_scalar_mul(pb[:, 1:5], s2raw[:, :], k2)
                        nc.vector.tensor_scalar_mul(pb[:, 5:14], s3raw[:, :], k3)
                        nc.vector.tensor_scalar_mul(pb[:, 14:50], A[:, :], k6)

                        featbf.append(fb)
                        poolbf.append(pb)

                    # main matmuls: out[p, j*50+n] = sum_c feat[c, p*72+j]*pool[c, n]
                    pss = []
                    if chsplit:
                        # One start=True per PSUM bank: marks the bank
                        # pending-zero; each region is zeroed on first touch,
                        # later writes accumulate. This lets every ch=0 matmul
                        # dispatch while the ch=1 input half is still
                        # streaming; only ch=1 matmuls sit on the tail.
                        for g in range(NBANK):
                            ps = psum_pool.tile([128, BANK_J * N], FP32, tag="ps")
                            pss.append(ps)
                            for k in range(BANK_J):
                                j = g * BANK_J + k
                                nc.tensor.matmul(
                                    ps[:, k * N:(k + 1) * N],
                                    featbf[0][:, j:j + JN * 127 + 1:JN],
                                    poolbf[0][:, :],
                                    start=(k == 0),
                                    stop=False,
                                )
                        for g in range(NBANK):
                            ps = pss[g]
                            for k in range(BANK_J):
                                j = g * BANK_J + k
                                nc.tensor.matmul(
                                    ps[:, k * N:(k + 1) * N],
                                    featbf[1][:, j:j + JN * 127 + 1:JN],
                                    poolbf[1][:, :],
                                    start=False,
                                    stop=(k == BANK_J - 1),
                                )
                    else:
                        for g in range(NBANK):
                            ps = psum_pool.tile([128, BANK_J * N], FP32, tag="ps")
                            pss.append(ps)
                            for k in range(BANK_J):
                                j = g * BANK_J + k
                                for ch in range(2):
                                    nc.tensor.matmul(
                                        ps[:, k * N:(k + 1) * N],
                                        featbf[ch][:, j:j + JN * 127 + 1:JN],
                                        poolbf[ch][:, :],
                                        start=(ch == 0),
                                        stop=(ch == 1),
                                    )

                    # relu PSUM -> SBUF (per bank), rowsum per bank, then a
                    # batched scale chain per `scale_group` banks, scaled
                    # output written as fp16 and DMA'd per `out_group` banks.
                    outb = outb_pool.tile([128, JN * N], FP32, tag="outb")
                    out_dram = out[s].rearrange("(p j) n -> p (j n)", p=128)
                    total = stats_pool.tile([128, JN], FP32, tag="total")
                    scale = stats_pool.tile([128, JN], FP32, tag="scale")
                    obf = outf_pool.tile([128, JN * N], out_dtype, tag="obf")
                    for g in range(NBANK):
                        bs = slice(g * BANK_J * N, (g + 1) * BANK_J * N)
                        # PSUM can only be read by ACT/DVE/PE (GPSIMD is
                        # forbidden by the BIR verifier). "split" alternates
                        # relu between DVE and ACT to keep ACT free for the
                        # input casts; the final sample keeps ACT only (DVE
                        # paces the drain there).
                        on_dve = (relu_eng == "vector"
                                  or (relu_eng == "split" and g % 2 == 0))
                        if last and rep == reps - 1:
                            on_dve = False
                        if on_dve:
                            nc.vector.tensor_scalar_max(outb[:, bs], pss[g][:, :], 0.0)
                        else:
                            nc.scalar.activation(
                                outb[:, bs], pss[g][:, :],
                                mybir.ActivationFunctionType.Relu,
                            )
                        nc.vector.reduce_sum(
                            total[:, g * BANK_J:(g + 1) * BANK_J],
                            outb[:, bs].rearrange("p (j n) -> p j n", n=N),
                            axis=mybir.AxisListType.X,
                        )
                        if g % scale_group == scale_group - 1:
                            js = slice((g + 1 - scale_group) * BANK_J,
                                       (g + 1) * BANK_J)
                            sq = stats_pool.tile([128, scale_group * BANK_J],
                                                 FP32, tag="sq")
                            nc.vector.tensor_mul(sq[:, :], total[:, js], total[:, js])
                            nc.vector.tensor_scalar_add(sq[:, :], sq[:, :], 1e-6)
                            nc.vector.reciprocal(scale[:, js], sq[:, :])
                            for gg in range(g + 1 - scale_group, g + 1):
                                ggs = slice(gg * BANK_J * N, (gg + 1) * BANK_J * N)
                                jj = slice(gg * BANK_J, (gg + 1) * BANK_J)
                                # mults on GpSimd free the loaded DVE; for the
                                # final sample's later banks use DVE (its
                                # chains have drained by then).
                                mul_eng = (nc.vector
                                           if (last and rep == reps - 1
                                               and gg >= NBANK // 2)
                                           else nc.gpsimd)
                                mul_eng.tensor_mul(
                                    obf[:, ggs].rearrange("p (j n) -> p j n", n=N),
                                    outb[:, ggs].rearrange("p (j n) -> p j n", n=N),
                                    scale[:, jj].unsqueeze(2).broadcast_to(
                                        (128, BANK_J, N)),
                                )
                        if g % out_group == out_group - 1:
                            os_ = slice((g + 1 - out_group) * BANK_J * N,
                                        (g + 1) * BANK_J * N)
                            getattr(nc, out_eng).dma_start(
                                out=out_dram[:, os_], in_=obf[:, os_])

    nc.compile()
    return nc


_NC_CACHE = None


def kernel(**inputs) -> np.ndarray:
    global _NC_CACHE
    x = np.ascontiguousarray(np.asarray(inputs["x"], dtype=np.float32))
    assert x.shape == (B, C, H, W)
    if _NC_CACHE is None:
        _NC_CACHE = build_nc()
    nc = _NC_CACHE
    in_maps = [{"x": x[i * BS:(i + 1) * BS]} for i in range(NCORES)]
    res = run_bass_kernel_spmd(nc, in_maps, list(range(NCORES)))
    outs = [res.results[i]["out"] for i in range(NCORES)]
    return np.concatenate(outs, axis=0).astype(np.float32)


if __name__ == "__main__":
    xt = np.random.randn(B, C, H, W).astype(np.float32)
    y = kernel(x=xt)
    print(y.shape, y.dtype)


# revision 9
# speedup vs baseline: 1.1619x; 1.1619x over previous
"""Trainium2 Bass kernel for nn_Cal_adj_matrix (pyramid-pool adjacency).

Computes, per sample b:
    feature = x[b].reshape(C, M)                  # M = H*W = 9216
    pool    = pyramid_pool(x[b])                  # (C, 50), pools of size 1,2,3,6
    sim     = relu(feature^T @ pool / (B*C*H*W))  # (M, 50)
    total   = sim.sum(-1)                         # (M,)
    adj     = sim / (total^2 + 1e-6)              # (M, 50)

Sharding: data-parallel over batch; 32 samples -> 4 per core x 8 cores.

Perf structure (memory-bound, ~360 GB/s/core DMA is the roofline):
  - input is read once as fp32 (37.7 MB/core), output written as fp16
    (3.7 MB/core) and upcast on host -> DMA floor ~115 us/core/pass.
  - fp32->bf16 cast on ACT; pyramid stage-1 (sum of 16 w-neighbours) as a
    bf16 pairwise-add tree on DVE (2x perf mode) instead of a 1x reduce.
  - matmuls accumulate both 128-channel halves into PSUM banks with a
    single start=True per bank (byte-granular lazy zeroing), so the ch0
    half's matmuls dispatch while the ch1 half is still streaming in.
  - m-index mapping m = p*72 + j keeps each output DMA contiguous.
"""

import numpy as np

import concourse.bass as bass
import concourse.bacc as bacc
import concourse.mybir as mybir
import concourse.tile as tile
from concourse.bass_utils import run_bass_kernel_spmd

# Problem shape (hardcoded; kernel.py must be self-contained).
B, C, H, W = 32, 256, 96, 96
M = H * W            # 9216
N = 50               # 1 + 4 + 9 + 36 pyramid tokens
NCORES = 8
BS = B // NCORES     # 4 samples per core
DIV = float(B * C * H * W)  # reference's global divisor

FP32 = mybir.dt.float32
BF16 = mybir.dt.bfloat16
FP16 = mybir.dt.float16

# m-index mapping: m = p*72 + j  (p = SBUF/PSUM partition, j = matmul index).
# This makes each sample's output a fully-contiguous DMA per partition line.
JN = M // 128        # 72 matmul column-groups per sample

BANK_J = 9           # matmul groups per PSUM bank (9*50=450 <= 512)
NBANK = JN // BANK_J  # 8 bank groups per sample


def build_nc(reps=1, xq_bufs=6, feat_bufs=3, outb_bufs=2, nq=4,
             out_dtype=FP16, stage1="tree", chsplit=True, scale_group=2,
             out_ring="gpsimd", out_group=2, relu_eng="split", in_rings=1):
    QH = H // nq         # h-rows per input chunk
    QM = QH * W          # elements per chunk
    nc = bacc.Bacc(
        "TRN2",
        target_bir_lowering=False,
        debug=False,
        enable_asserts=True,
        num_devices=NCORES,
    )
    x = nc.dram_tensor("x", [BS, C, H, W], FP32, kind="ExternalInput").ap()
    out = nc.dram_tensor("out", [BS, M, N], out_dtype, kind="ExternalOutput").ap()

    # scale factors folded into the pool values: 1/(bin_elems * DIV)
    k1 = 1.0 / (9216.0 * DIV)
    k2 = 1.0 / (2304.0 * DIV)
    k3 = 1.0 / (1024.0 * DIV)
    k6 = 1.0 / (256.0 * DIV)

    out_eng = {"scalar": "scalar", "gpsimd": "gpsimd", "vector": "vector",
               "sync": "sync"}[out_ring]

    with tile.TileContext(nc) as tc:
        with (
            tc.tile_pool(name="xq", bufs=xq_bufs) as xq_pool,
            tc.tile_pool(name="featbf", bufs=feat_bufs) as feat_pool,
            tc.tile_pool(name="tree", bufs=4) as tree_pool,
            tc.tile_pool(name="r1", bufs=4) as r1_pool,
            tc.tile_pool(name="pools", bufs=8) as small_pool,
            tc.tile_pool(name="poolbf", bufs=4) as poolbf_pool,
            tc.tile_pool(name="outb", bufs=outb_bufs) as outb_pool,
            tc.tile_pool(name="outf", bufs=4) as outf_pool,
            tc.tile_pool(name="stats", bufs=4) as stats_pool,
            tc.tile_pool(name="psum", bufs=8, space="PSUM") as psum_pool,
        ):
            for rep in range(reps):
                for s in range(BS):
                    last = s == BS - 1
                    featbf = []
                    poolbf = []
                    for ch in range(2):
                        c0 = ch * 128
                        fb = feat_pool.tile([128, M], BF16, tag="featbf")
                        r1 = r1_pool.tile([128, 576], FP32, tag="r1")
                        for q in range(nq):
                            h0 = q * QH
                            t32 = xq_pool.tile([128, QM], FP32, tag="xq")
                            src = x[s, c0:c0 + 128, h0:h0 + QH, :]
                            # optionally alternate input chunks across two
                            # HWDGE rings so descriptor processing parallelizes
                            in_eng = (nc.scalar
                                      if (in_rings == 2 and (ch * nq + q) % 2)
                                      else nc.sync)
                            in_eng.dma_start(
                                out=t32[:], in_=src.rearrange("c h w -> c (h w)"))
                            # fp32 -> bf16 cast on ScalarE
                            fbs = fb[:, h0 * W:(h0 + QH) * W]
                            nc.scalar.copy(fbs, t32[:])
                            r1s = r1[:, h0 * 6:(h0 + QH) * 6]
                            # stage-1 pool: sum 16 contiguous w-elements.
                            # "tree": bf16 pairwise adds on DVE (2x mode);
                            # for the tail-critical final chunk, reduce from
                            # the fp32 tile instead so it runs concurrently
                            # with the cast.
                            tail_chunk = (last and ch == 1 and q == nq - 1
                                          and rep == reps - 1)
                            if stage1 == "tree" and not tail_chunk:
                                g = QH * 6
                                a = fbs.rearrange("p (g k) -> p g k", k=16)
                                t1 = tree_pool.tile([128, g * 8], BF16, tag="t1")
                                v1 = t1[:, :].rearrange("p (g k) -> p g k", k=8)
                                nc.vector.tensor_add(v1, a[:, :, 0:8], a[:, :, 8:16])
                                t2 = tree_pool.tile([128, g * 4], BF16, tag="t2")
                                v2 = t2[:, :].rearrange("p (g k) -> p g k", k=4)
                                nc.vector.tensor_add(v2, v1[:, :, 0:4], v1[:, :, 4:8])
                                t3 = tree_pool.tile([128, g * 2], BF16, tag="t3")
                                v3 = t3[:, :].rearrange("p (g k) -> p g k", k=2)
                                nc.vector.tensor_add(v3, v2[:, :, 0:2], v2[:, :, 2:4])
                                nc.vector.tensor_add(r1s, v3[:, :, 0], v3[:, :, 1])
                            else:
                                nc.vector.reduce_sum(
                                    r1s,
                                    t32[:, :].rearrange("p (g k) -> p g k", k=16),
                                    axis=mybir.AxisListType.X,
                                )
                        # stage-2: A[hb,wb] = 16x16 block sums.  r1 free idx =
                        # h*6+wb, h = hb*16+hh  ->  idx = hb*96 + hh*6 + wb
                        A = small_pool.tile([128, 36], FP32, tag="A")
                        nc.vector.reduce_sum(
                            A[:, :],
                            r1[:, :576].rearrange(
                                "p (hb hh wb) -> p hb wb hh", hb=6, hh=16, wb=6),
                            axis=mybir.AxisListType.X,
                        )
                        # s=3 pools: 2x2 groups of A blocks
                        Bt = small_pool.tile([128, 18], FP32, tag="B")
                        a2 = A[:, :36].rearrange(
                            "p (hb wp t) -> p t hb wp", hb=6, wp=3, t=2)
                        nc.vector.tensor_add(Bt[:, :], a2[:, 0, :], a2[:, 1, :])
                        s3raw = small_pool.tile([128, 9], FP32, tag="s3")
                        b2 = Bt[:, :18].rearrange(
                            "p (hp t wp) -> p t hp wp", hp=3, t=2, wp=3)
                        nc.vector.tensor_add(s3raw[:, :], b2[:, 0, :], b2[:, 1, :])
                        # s=2 pools: 3x3 groups of A blocks
                        Ct = small_pool.tile([128, 12], FP32, tag="C")
                        nc.vector.reduce_sum(
                            Ct[:, :],
                            A[:, :36].rearrange(
                                "p (hb wq wt) -> p (hb wq) wt", hb=6, wq=2, wt=3),
                            axis=mybir.AxisListType.X,
                        )
                        s2raw = small_pool.tile([128, 4], FP32, tag="s2")
                        nc.vector.reduce_sum(
                            s2raw[:, :],
                            Ct[:, :12].rearrange(
                                "p (hq ht wq) -> p hq wq ht", hq=2, ht=3, wq=2),
                            axis=mybir.AxisListType.X,
                        )
                        # s=1 pool
                        s1raw = small_pool.tile([128, 1], FP32, tag="s1")
                        nc.vector.reduce_sum(
                            s1raw[:, :], A[:, :36], axis=mybir.AxisListType.X)

                        pb = poolbf_pool.tile([128, N], BF16, tag="poolbf")
                        nc.vector.tensor_scalar_mul(pb[:, 0:1], s1raw[:, :], k1)
                        nc.vector.tensor_scalar_mul(pb[:, 1:5], s2raw[:, :], k2)
                        nc.vector.tensor_scalar_mul(pb[:, 5:14], s3raw[:, :], k3)
                        nc.vector.tensor_scalar_mul(pb[:, 14:50], A[:, :], k6)

                        featbf.append(fb)
                        poolbf.append(pb)

                    # main matmuls: out[p, j*50+n] = sum_c feat[c, p*72+j]*pool[c, n]
                    pss = []
                    if chsplit:
                        # One start=True per PSUM bank: marks the bank
                        # pending-zero; each region is zeroed on first touch,
                        # later writes accumulate. This lets every ch=0 matmul
                        # dispatch while the ch=1 input half is still
                        # streaming; only ch=1 matmuls sit on the tail.
                        for g in range(NBANK):
                            ps = psum_pool.tile([128, BANK_J * N], FP32, tag="ps")
                            pss.append(ps)
                            for k in range(BANK_J):
                                j = g * BANK_J + k
                                nc.tensor.matmul(
                                    ps[:, k * N:(k + 1) * N],
                                    featbf[0][:, j:j + JN * 127 + 1:JN],
                                    poolbf[0][:, :],
                                    start=(k == 0),
                                    stop=False,
                                )
                        for g in range(NBANK):
                            ps = pss[g]
                            for k in range(BANK_J):
                                j = g * BANK_J + k
                                nc.tensor.matmul(
                                    ps[:, k * N:(k + 1) * N],
                                    featbf[1][:, j:j + JN * 127 + 1:JN],
                                    poolbf[1][:, :],
                                    start=False,
                                    stop=(k == BANK_J - 1),
                                )
                    else:
                        for g in range(NBANK):
                            ps = psum_pool.tile([128, BANK_J * N], FP32, tag="ps")
                            pss.append(ps)
                            for k in range(BANK_J):
                                j = g * BANK_J + k
                                for ch in range(2):
                                    nc.tensor.matmul(
                                        ps[:, k * N:(k + 1) * N],
                                        featbf[ch][:, j:j + JN * 127 + 1:JN],
                                        poolbf[ch][:, :],
                                        start=(ch == 0),
                                        stop=(ch == 1),
                                    )

                    # relu PSUM -> SBUF (per bank), rowsum per bank, then a
                    # batched scale chain per `scale_group` banks, scaled
                    # output written as fp16 and DMA'd per `out_group` banks.
                    outb = outb_pool.tile([128, JN * N], FP32, tag="outb")
                    out_dram = out[s].rearrange("(p j) n -> p (j n)", p=128)
                    total = stats_pool.tile([128, JN], FP32, tag="total")
                    scale = stats_pool.tile([128, JN], FP32, tag="scale")
                    obf = outf_pool.tile([128, JN * N], out_dtype, tag="obf")
                    for g in range(NBANK):
                        bs = slice(g * BANK_J * N, (g + 1) * BANK_J * N)
                        # PSUM can only be read by ACT/DVE/PE (GPSIMD is
                        # forbidden by the BIR verifier). "split" alternates
                        # relu between DVE and ACT to keep ACT free for the
                        # input casts; the final sample keeps ACT only (DVE
                        # paces the drain there).
                        on_dve = (relu_eng == "vector"
                                  or (relu_eng == "split" and g % 2 == 0))
                        if last and rep == reps - 1:
                            on_dve = False
                        if on_dve:
                            nc.vector.tensor_scalar_max(outb[:, bs], pss[g][:, :], 0.0)
                        else:
                            nc.scalar.activation(
                                outb[:, bs], pss[g][:, :],
                                mybir.ActivationFunctionType.Relu,
                            )
                        nc.vector.reduce_sum(
                            total[:, g * BANK_J:(g + 1) * BANK_J],
                            outb[:, bs].rearrange("p (j n) -> p j n", n=N),
                            axis=mybir.AxisListType.X,
                        )
                        if g % scale_group == scale_group - 1:
                            js = slice((g + 1 - scale_group) * BANK_J,
                                       (g + 1) * BANK_J)
                            sq = stats_pool.tile([128, scale_group * BANK_J],
                                                 FP32, tag="sq")
                            nc.vector.tensor_mul(sq[:, :], total[:, js], total[:, js])
                            nc.vector.tensor_scalar_add(sq[:, :], sq[:, :], 1e-6)
                            nc.vector.reciprocal(scale[:, js], sq[:, :])
                            for gg in range(g + 1 - scale_group, g + 1):
                                ggs = slice(gg * BANK_J * N, (gg + 1) * BANK_J * N)
                                jj = slice(gg * BANK_J, (gg + 1) * BANK_J)
                                # mults on GpSimd free the loaded DVE; for the
                                # final sample's later banks use DVE (its
                                # chains have drained by then).
                                mul_eng = (nc.vector
                                           if (last and rep == reps - 1
                                               and gg >= NBANK // 2)
                                           else nc.gpsimd)
                                mul_eng.tensor_mul(
                                    obf[:, ggs].rearrange("p (j n) -> p j n", n=N),
                                    outb[:, ggs].rearrange("p (j n) -> p j n", n=N),
                                    scale[:, jj].unsqueeze(2).broadcast_to(
                                        (128, BANK_J, N)),
                                )
                        if g % out_group == out_group - 1:
                            os_ = slice((g + 1 - out_group) * BANK_J * N,
                                        (g + 1) * BANK_J * N)
                            getattr(nc, out_eng).dma_start(
                                out=out_dram[:, os_], in_=obf[:, os_])

    nc.compile()
    return nc


_NC_CACHE = None


def kernel(**inputs) -> np.ndarray:
    global _NC_CACHE
    x = np.ascontiguousarray(np.asarray(inputs["x"], dtype=np.float32))
    assert x.shape == (B, C, H, W)
    if _NC_CACHE is None:
        _NC_CACHE = build_nc()
    nc = _NC_CACHE
    in_maps = [{"x": x[i * BS:(i + 1) * BS]} for i in range(NCORES)]
    res = run_bass_kernel_spmd(nc, in_maps, list(range(NCORES)))
    outs = [res.results[i]["out"] for i in range(NCORES)]
    return np.concatenate(outs, axis=0).astype(np.float32)


if __name__ == "__main__":
    xt = np.random.randn(B, C, H, W).astype(np.float32)
    y = kernel(x=xt)
    print(y.shape, y.dtype)
